# revision 1
# baseline (speedup 1.0000x reference)
"""Trainium2 Bass kernel: full 1-D convolution (2,097,152-sample signal with a
32,000-tap RIR) followed by peak-normalization, distributed over 8 NeuronCores.

Algorithm: block-Toeplitz formulation of the convolution.
  y[128q + r] = sum_c sum_s  h[128c + r - s] * x_{q-c}[s]
i.e. y_q = sum_{c=0}^{250} M_c^T x_{q-c}, where M_c[s, r] = h[128c + r - s]
(zero outside [0, 32000)).  Each M_c is a 128x128 Toeplitz matrix built on the
host from the RIR; the x blocks live as columns of an SBUF-resident [128, nblk]
matrix, so the tensor engine runs 251 accumulating matmuls per output tile with
the rhs being a sliding column window of the same SBUF tensor.

Sequence-parallel across 8 cores: core i computes output blocks
[2080 i, 2080 (i+1)) and receives its input span plus a 250-block left halo.
The peak-normalize max is combined with an on-device AllReduce(max) and the
scale (1 / max(m, 1)) is applied before the output leaves the device.

Matmuls run in float32r (TF32-like, full bf16-rate on the PE at free dim >=
256) with fp32 PSUM accumulation.
"""
import numpy as np

B = 128                      # block size / partition count
KLEN = 32_000                # RIR taps
N = 2_097_152                # signal samples
NOUT = N + KLEN - 1          # full-convolution output length
C = 251                      # number of 128-tap Toeplitz chunks
HALO = C - 1                 # left halo in blocks
NCORES = 8
BLK_PER_CORE = 2080          # output blocks per core (8*2080 = 16640 >= 16634)
LOCAL_IN = BLK_PER_CORE + HALO   # input blocks per core (2330)
F = 416                      # matmul moving free dim (psum tile columns)
NTILES = BLK_PER_CORE // F   # 5 psum tiles per core

_NC_CACHE = None


def _build_nc(collective=True, passes=1):
    import concourse.bacc as bacc
    import concourse.bass as bass
    import concourse.mybir as mybir
    from concourse import tile

    f32 = mybir.dt.float32
    f32r = mybir.dt.float32r

    nc = bacc.Bacc("TRN2", target_bir_lowering=False, debug=False,
                   num_devices=NCORES)

    x_in = nc.dram_tensor("x", [B, LOCAL_IN], f32r, kind="ExternalInput")
    w_in = nc.dram_tensor("w", [B, C * B], f32r, kind="ExternalInput")
    y_out = nc.dram_tensor("y", [B, BLK_PER_CORE], f32, kind="ExternalOutput")

    with tile.TileContext(nc) as tc:
        with (
            tc.tile_pool(name="data", bufs=1) as data_pool,
            tc.tile_pool(name="wpool", bufs=1) as wpool,
            tc.tile_pool(name="ps", bufs=1, space="PSUM") as ps_pool,
            tc.tile_pool(name="dram", bufs=1, space="DRAM") as dram_pool,
        ):
            x_sb = data_pool.tile([B, LOCAL_IN], f32r, name="x_sb")
            y_sb = data_pool.tile([B, BLK_PER_CORE], f32, name="y_sb")
            mx = data_pool.tile([B, NTILES], f32, name="mx")
            w_sb = wpool.tile([B, C * B], f32r, name="w_sb")

            # warm the PE (HAM clock gate) during the initial DMA wait with
            # dummy matmuls on a zeroed tile — no data dependencies
            warm = data_pool.tile([B, 512], mybir.dt.bfloat16, name="warm")
            nc.gpsimd.memset(warm[:], 0.0)
            wps = ps_pool.tile([B, 512], f32, name="wps", tag="wps")
            for _ in range(12):
                nc.tensor.matmul(wps[:], warm[:, :B], warm[:, :512],
                                 start=True, stop=True)

            # tile 0 reads x cols [0, HALO+F); load those first (on a separate
            # engine queue from the weights) so the PE can start early.
            XSPLIT = HALO + F
            nc.scalar.dma_start(x_sb[:, :XSPLIT], x_in[:, :XSPLIT])
            # weights in batches, in consumption order (c ascending), small
            # batches first so chunk delivery keeps pace with the tile-0
            # c-loop from the start
            ws = [4, 8, 16]
            while sum(ws) < C:
                ws.append(min(16, C - sum(ws)))
            b0 = 0
            for i, wn in enumerate(ws):
                b1 = b0 + wn
                nc.sync.dma_start(w_sb[:, b0 * B:b1 * B], w_in[:, b0 * B:b1 * B])
                if i == 0:
                    nc.scalar.dma_start(x_sb[:, XSPLIT:], x_in[:, XSPLIT:])
                b0 = b1

            def one_pass():
                # Phase 1, chunk-major over the first C1 chunks: all NTILES
                # psum banks accumulate concurrently, so each weight chunk is
                # consumed NTILES matmuls in a row (~5x134 ns) — faster than
                # HBM delivers chunks (~178 ns), so the PE never stalls on the
                # weight stream.  By chunk C1 the whole weight tensor is
                # resident.  Phase 2, tile-major: tiles finish staggered, so
                # their reduces/copies hide under the next tile's matmuls and
                # only the last tile's reduce lands in the kernel tail.
                C1 = 72
                pss = [ps_pool.tile([B, F], f32, name=f"ps{t}", tag=f"ps{t}")
                       for t in range(NTILES)]

                def mm(t, c):
                    lo = t * F + HALO - c
                    nc.tensor.matmul(
                        pss[t][:],
                        w_sb[:, c * B:(c + 1) * B],
                        x_sb[:, lo:lo + F],
                        start=(c == 0),
                        stop=(c == C - 1),
                    )

                for c in range(C1):
                    for t in range(NTILES):
                        mm(t, c)
                for t in range(NTILES):
                    for c in range(C1, C):
                        mm(t, c)
                    nc.vector.tensor_reduce(
                        mx[:, t:t + 1], pss[t][:], axis=mybir.AxisListType.X,
                        op=mybir.AluOpType.max, apply_absolute_value=True,
                    )
                    if t < NTILES - 1:
                        # hidden under the next tile's phase-2 matmuls
                        nc.vector.tensor_copy(y_sb[:, t * F:(t + 1) * F],
                                              pss[t][:])

                # local scalar max -> all partitions
                am = data_pool.tile([B, 1], f32, name="am")
                nc.vector.tensor_reduce(
                    am[:], mx[:], axis=mybir.AxisListType.X,
                    op=mybir.AluOpType.max,
                )
                gm = data_pool.tile([B, 1], f32, name="gm")
                nc.gpsimd.partition_all_reduce(
                    gm[:], am[:], B, bass.bass_isa.ReduceOp.max
                )

                scb = data_pool.tile([B, 1], f32, name="scb")
                if collective:
                    # global max across the 8 cores
                    cc_in = dram_pool.tile([B, 1], f32, name="cc_in")
                    cc_out = dram_pool.tile([B, 1], f32, name="cc_out",
                                            addr_space="Shared")
                    nc.sync.dma_start(cc_in[:], gm[:])
                    nc.gpsimd.collective_compute(
                        "AllReduce",
                        mybir.AluOpType.max,
                        replica_groups=[list(range(NCORES))],
                        ins=[cc_in[:].opt()],
                        outs=[cc_out[:].opt()],
                    )
                    nc.sync.dma_start(scb[:], cc_out[:])
                else:
                    # single-core variant for TimelineSim (no collectives)
                    nc.vector.tensor_copy(scb[:], gm[:])

                # y *= 1 / max(m, 1); chunked so the store DMAs (on two
                # queues) overlap the scaling
                nc.vector.tensor_scalar_max(scb[:], scb[:], 1.0)
                nc.vector.reciprocal(scb[:], scb[:])
                for t in range(NTILES):
                    sl = slice(t * F, (t + 1) * F)
                    src = pss[t][:] if t == NTILES - 1 else y_sb[:, sl]
                    nc.vector.tensor_scalar_mul(y_sb[:, sl], src, scb[:, 0:1])
                    eng = nc.sync if t % 2 == 0 else nc.scalar
                    eng.dma_start(y_out[:, sl], y_sb[:, sl])

            for _ in range(passes):  # passes > 1 only for wall-clock timing
                one_pass()

    nc.compile()
    return nc


def _build_weights(h):
    """[128, C*128] f32: column block c holds M_c with M_c[s, r] = h[128c+r-s]."""
    hp = np.zeros(B * (C - 1) + 2 * B, np.float32)
    hp[B - 1:B - 1 + KLEN] = h
    idx = (B - 1) + np.arange(B)[None, :] - np.arange(B)[:, None]  # [s, r]
    offs = B * np.arange(C)
    W = hp[offs[:, None, None] + idx[None, :, :]]                  # [C, s, r]
    return np.ascontiguousarray(W.transpose(1, 0, 2).reshape(B, C * B))


def _shard(data, i):
    """Core i's input: blocks [q0-HALO, q0+BLK_PER_CORE) as a [128, LOCAL_IN]
    matrix (column q = block q), zero-padded outside [0, N)."""
    q0 = i * BLK_PER_CORE
    lo = (q0 - HALO) * B
    hi = (q0 + BLK_PER_CORE) * B
    seg = np.zeros(hi - lo, np.float32)
    s0, s1 = max(lo, 0), min(hi, N)
    if s1 > s0:
        seg[s0 - lo:s1 - lo] = data[s0:s1]
    return np.ascontiguousarray(seg.reshape(LOCAL_IN, B).T)


def kernel(data, rir):
    global _NC_CACHE
    from concourse.bass_utils import run_bass_kernel_spmd

    data = np.asarray(data, dtype=np.float32).reshape(-1)
    h = np.asarray(rir, dtype=np.float32).reshape(-1)

    if _NC_CACHE is None:
        _NC_CACHE = _build_nc()
    nc = _NC_CACHE

    w = _build_weights(h)
    in_maps = [{"x": _shard(data, i), "w": w} for i in range(NCORES)]
    res = run_bass_kernel_spmd(nc, in_maps, core_ids=list(range(NCORES)))

    y = np.empty(NCORES * BLK_PER_CORE * B, np.float32)
    span = BLK_PER_CORE * B
    for i in range(NCORES):
        y[i * span:(i + 1) * span] = res.results[i]["y"].T.reshape(-1)
    return y[:NOUT]



# revision 4
# speedup vs baseline: 1.0577x; 1.0577x over previous
"""Trainium2 Bass kernel: full 1-D convolution (2,097,152 samples x 32,000-tap
RIR) + peak-normalization on 8 NeuronCores.

v3 = v2 (fp8 e4m3 DoubleRow, 3-product precision split) + one level of
odd-even Karatsuba over 128-sample blocks:

  blocks split by parity:  xe_k = x_{2k}, xo_k = x_{2k+1};
  chunks split by parity:  Me_g = M_{2g}, Mo_g = M_{2g+1}  (126 each).

  A = Me*xe, B = Mo*xo, S = (Me+Mo)*(xe+xo)      (3 half-length sub-convs)
  y_{2m}   = A[m] + B[m-1]
  y_{2m+1} = S[m] - A[m] - B[m]

This does the 252-chunk convolution with 3 x 126-chunk sub-convolutions over
half-length sequences: 25% fewer PE column-units.  Each sub-conv keeps the
v2 structure: DoubleRow chunk pairs (2g, 2g+1) via two shifted x planes, and
the 3-product fp8 precision split (hi*hi + lo*hi + hi*lo) at a common scale.

B is accumulated shifted by one (B_ps[m'] = B[m'-1]) so the even-output
combine is column-aligned; the odd-output combine reads B_ps[m'+1].  The xo
operand's host layout carries one extra left shift so every DoubleRow moving
window keeps an even byte offset (hardware requirement discovered in v2:
odd offsets/strides on fp8 DR moving operands crash the device).

Combine + abs-max run on the DVE, hidden under the next group's matmuls.
"""
import numpy as np

B = 128                      # block size / partition count
KLEN = 32_000                # RIR taps
N = 2_097_152                # signal samples
NOUT = N + KLEN - 1          # full-convolution output length
NSUB = 126                   # chunks per sub-convolution (parity split of 252)
G = NSUB // 2                # 63 DoubleRow chunk pairs per sub-conv
NCORES = 8
BLK_PER_CORE = 2080          # output blocks per core
HB_PER_CORE = 1040           # output half-blocks (polyphase) per core
XW = 1170                    # x operand width per core (cols, even)
XPAD = 128                   # so window lo = XPAD + base - 2g stays >= 4, even
GROUPS = [(0, 416), (416, 416), (832, 208)]   # (base, Fw) psum tile groups
SX = 16.0                    # x pre-quant scale
SH = 1024.0                  # h pre-quant scale

_NC_CACHE = None


def _build_nc(collective=True, passes=1, dbg_skip_combine=False,
              dbg_skip_tail=False, dbg_nwarm=8, dbg_split_x=2,
              dbg_no_par=False, dbg_no_scale=False, dbg_scale_mode=0,
              dbg_ws=(2, 2, 4, 8, 16)):
    import concourse.bacc as bacc
    import concourse.bass as bass
    import concourse.mybir as mybir
    from concourse import tile

    f32 = mybir.dt.float32
    f8 = mybir.dt.float8e4
    DR = mybir.MatmulPerfMode.DoubleRow
    add = mybir.AluOpType.add
    sub = mybir.AluOpType.subtract

    nc = bacc.Bacc("TRN2", target_bir_lowering=False, debug=False,
                   num_devices=NCORES)

    # x: (op: 0=xe 1=xo 2=xs, stream hi/lo, plane, col)
    x_in = nc.dram_tensor("x", [B, 3, 2, 2, XW], f8, kind="ExternalInput")
    # w: (pair g, sub-conv 0=Me 1=Mo 2=Msum, stream hi/lo, ktile, r)
    w_in = nc.dram_tensor("w", [B, G, 3, 2, 2, B], f8, kind="ExternalInput")
    y_out = nc.dram_tensor("y", [B, HB_PER_CORE, 2], f32, kind="ExternalOutput")

    with tile.TileContext(nc) as tc:
        with (
            tc.tile_pool(name="data", bufs=1) as data_pool,
            tc.tile_pool(name="wpool", bufs=1) as wpool,
            tc.tile_pool(name="ps", bufs=1, space="PSUM") as ps_pool,
            tc.tile_pool(name="dram", bufs=1, space="DRAM") as dram_pool,
        ):
            x_sb = data_pool.tile([B, 3, 2, 2, XW], f8, name="x_sb")
            y_sb = data_pool.tile([B, HB_PER_CORE, 2], f32, name="y_sb")
            mx = data_pool.tile([B, 4], f32, name="mx")
            w_sb = wpool.tile([B, G, 3, 2, 2, B], f8, name="w_sb")
            a_sb = data_pool.tile([B, 416], f32, name="a_sb")
            t1_sb = data_pool.tile([B, 416], f32, name="t1_sb")

            # warm the PE (p-state ramp) during the initial DMA wait
            warm = data_pool.tile([B, 512], mybir.dt.bfloat16, name="warm")
            nc.gpsimd.memset(warm[:], 0.0)
            wps = ps_pool.tile([B, 256], f32, name="wps", tag="wps")
            for _ in range(dbg_nwarm):
                nc.tensor.matmul(wps[:], warm[:, :B], warm[:, :256],
                                 start=True, stop=True)

            if dbg_split_x == 1:
                nc.scalar.dma_start(x_sb[:, 0], x_in[:, 0])
                nc.gpsimd.dma_start(x_sb[:, 1], x_in[:, 1])
                nc.gpsimd.dma_start(x_sb[:, 2], x_in[:, 2])
            elif dbg_split_x == 2:
                # per-op DMAs on one queue: each op's completion unblocks its
                # sweep (A after ~1.7us, B after ~3.3, S after ~5.0)
                for opi in range(3):
                    nc.scalar.dma_start(x_sb[:, opi], x_in[:, opi])
            else:
                nc.scalar.dma_start(x_sb[:], x_in[:])
            ws = list(dbg_ws)
            while sum(ws) < G:
                ws.append(min(16, G - sum(ws)))
            b0 = 0
            for wn in ws:
                b1 = b0 + wn
                nc.sync.dma_start(w_sb[:, b0:b1], w_in[:, b0:b1])
                b0 = b1

            def one_pass():
                for gi, (base, Fw) in enumerate(GROUPS):
                    pA = ps_pool.tile([B, 416], f32, name=f"pA{gi % 2}",
                                      tag=f"pA{gi % 2}")
                    pB = ps_pool.tile([B, 418], f32, name=f"pB{gi % 2}",
                                      tag=f"pB{gi % 2}")
                    pS = ps_pool.tile([B, 416], f32, name=f"pS{gi % 2}",
                                      tag=f"pS{gi % 2}")
                    FB = Fw + 2
                    # A/B sweep first, S sweep after: the S operand (xs) is
                    # last off the gpsimd DMA queue, so deferring S keeps the
                    # in-order PE from stalling on it at group start.
                    sweeps = (((0, pA, Fw),), ((1, pB, FB),), ((2, pS, Fw),)) \
                        if dbg_split_x == 2 else \
                        (((0, pA, Fw), (1, pB, FB)), ((2, pS, Fw),))
                    for sweep in sweeps:
                        for g in range(G):
                            lo = XPAD + base - 2 * g
                            for sb, ps, Fi in sweep:
                                rhs_hi = x_sb[:, sb, 0, :, lo:lo + Fi]
                                rhs_lo = x_sb[:, sb, 1, :, lo:lo + Fi]
                                first = (g == 0)
                                last = (g == G - 1)
                                out = ps[:, :Fi]
                                nc.tensor.matmul(out, w_sb[:, g, sb, 0],
                                                 rhs_hi, start=first,
                                                 stop=False, perf_mode=DR)
                                nc.tensor.matmul(out, w_sb[:, g, sb, 1],
                                                 rhs_hi, start=False,
                                                 stop=False, perf_mode=DR)
                                nc.tensor.matmul(out, w_sb[:, g, sb, 0],
                                                 rhs_lo, start=False,
                                                 stop=last, perf_mode=DR)

                    if dbg_skip_combine:
                        continue
                    # combine (DVE; hides under the next group's matmuls):
                    # ye = A + B_ps (same col), yo = S - A - B_ps[+1]
                    nc.vector.tensor_copy(a_sb[:, :Fw], pA[:, :Fw])
                    nc.vector.scalar_tensor_tensor(
                        y_sb[:, base:base + Fw, 0], a_sb[:, :Fw], 0.0,
                        pB[:, :Fw], op0=add, op1=add)
                    nc.vector.scalar_tensor_tensor(
                        t1_sb[:, :Fw], a_sb[:, :Fw], 0.0,
                        pB[:, 1:Fw + 1], op0=add, op1=add)
                    nc.vector.scalar_tensor_tensor(
                        y_sb[:, base:base + Fw, 1], pS[:, :Fw], 0.0,
                        t1_sb[:, :Fw], op0=add, op1=sub)
                    nc.vector.tensor_reduce(
                        mx[:, gi:gi + 1],
                        y_sb[:, base:base + Fw, :],
                        axis=mybir.AxisListType.XY,
                        op=mybir.AluOpType.max, apply_absolute_value=True,
                    )

                if dbg_skip_tail:
                    return
                # local scalar max -> all partitions
                am = data_pool.tile([B, 1], f32, name="am")
                nc.vector.tensor_reduce(
                    am[:], mx[:, :3], axis=mybir.AxisListType.X,
                    op=mybir.AluOpType.max,
                )
                gm = data_pool.tile([B, 1], f32, name="gm")
                if dbg_no_par:
                    nc.vector.tensor_copy(gm[:], am[:])
                else:
                    nc.gpsimd.partition_all_reduce(
                        gm[:], am[:], B, bass.bass_isa.ReduceOp.max
                    )

                scb = data_pool.tile([B, 1], f32, name="scb")
                if collective:
                    cc_in = dram_pool.tile([B, 1], f32, name="cc_in")
                    cc_out = dram_pool.tile([B, 1], f32, name="cc_out",
                                            addr_space="Shared")
                    nc.sync.dma_start(cc_in[:], gm[:])
                    nc.gpsimd.collective_compute(
                        "AllReduce",
                        mybir.AluOpType.max,
                        replica_groups=[list(range(NCORES))],
                        ins=[cc_in[:].opt()],
                        outs=[cc_out[:].opt()],
                    )
                    nc.sync.dma_start(scb[:], cc_out[:])
                else:
                    nc.vector.tensor_copy(scb[:], gm[:])

                # y_sb holds y*SX*SH; y_out = y_sb/max(gm, SX*SH)
                if dbg_no_scale:
                    return
                nc.vector.tensor_scalar_max(scb[:], scb[:], SX * SH)
                nc.vector.reciprocal(scb[:], scb[:])
                if dbg_scale_mode == 0:
                    for t in range(5):
                        sl = slice(t * 208, (t + 1) * 208)
                        nc.vector.tensor_scalar_mul(y_sb[:, sl, :],
                                                    y_sb[:, sl, :],
                                                    scb[:, 0:1])
                        eng = nc.sync if t % 2 == 0 else nc.scalar
                        eng.dma_start(y_out[:, sl, :], y_sb[:, sl, :])
                elif dbg_scale_mode == 1:
                    nc.vector.tensor_scalar_mul(y_sb[:], y_sb[:], scb[:, 0:1])
                    nc.sync.dma_start(y_out[:], y_sb[:])
                elif dbg_scale_mode == 2:
                    for t in range(10):
                        sl = slice(t * 104, (t + 1) * 104)
                        nc.vector.tensor_scalar_mul(y_sb[:, sl, :],
                                                    y_sb[:, sl, :],
                                                    scb[:, 0:1])
                        eng = nc.sync if t % 2 == 0 else nc.scalar
                        eng.dma_start(y_out[:, sl, :], y_sb[:, sl, :])
                elif dbg_scale_mode == 3:
                    # ACT engine does the scale (frees DVE), DMAs staggered
                    for t in range(5):
                        sl = slice(t * 208, (t + 1) * 208)
                        nc.scalar.mul(y_sb[:, sl, :], y_sb[:, sl, :],
                                      scb[:, 0:1])
                        deng = nc.sync if t % 2 == 0 else nc.scalar
                        deng.dma_start(y_out[:, sl, :], y_sb[:, sl, :])
                elif dbg_scale_mode == 4:
                    for t in range(5):     # DMA only (timing probe)
                        sl = slice(t * 208, (t + 1) * 208)
                        eng = nc.sync if t % 2 == 0 else nc.scalar
                        eng.dma_start(y_out[:, sl, :], y_sb[:, sl, :])
                elif dbg_scale_mode == 6:
                    # all muls issued first; DMAs trail on two queues
                    for t in range(5):
                        sl = slice(t * 208, (t + 1) * 208)
                        nc.vector.tensor_scalar_mul(y_sb[:, sl, :],
                                                    y_sb[:, sl, :],
                                                    scb[:, 0:1])
                    for t in range(5):
                        sl = slice(t * 208, (t + 1) * 208)
                        eng = nc.sync if t % 2 == 0 else nc.scalar
                        eng.dma_start(y_out[:, sl, :], y_sb[:, sl, :])


            for _ in range(passes):
                one_pass()

    nc.compile()
    return nc


def _q8(v):
    import ml_dtypes
    return np.clip(np.asarray(v, np.float32), -240.0, 240.0).astype(
        ml_dtypes.float8_e4m3)


def _toep256(ext, shift):
    """W[gamma, s, r] = ext[256*gamma + (r - s) + shift], gamma in [0, NSUB).
    ext[v] holds tap u = v - 128 (128-zero left pad), so shift=128 indexes
    taps at 256g + (r-s) and shift=256 at 256g + 128 + (r-s), keeping the
    left-edge (u < 0 from r<s at g=0) entries correct."""
    idx = shift + np.arange(B)[None, :] - np.arange(B)[:, None]
    offs = 256 * np.arange(NSUB)
    W = ext[offs[:, None, None] + idx[None, :, :]]          # [NSUB, s, r]
    return W.transpose(1, 0, 2)                             # [s, NSUB, r]


def _build_weights(h):
    """[B, G, 3(sub), 2(hi/lo), 2(ktile), B] fp8."""
    import ml_dtypes
    EXTLEN = 256 * (NSUB - 1) + 2 * B + 256                 # 32384
    ext = np.zeros(EXTLEN, np.float32)
    ext[B:B + KLEN] = h                                     # ext[v] = h[v-128]
    sum_ext = ext + np.concatenate([ext[B:], np.zeros(B, np.float32)])
    w = np.empty((B, G, 3, 2, 2, B), dtype=ml_dtypes.float8_e4m3)
    for sb, (taps, shift) in enumerate(
            ((ext, 128), (ext, 256), (sum_ext, 128))):
        th = _q8(taps * SH)
        tl = _q8(taps * SH - th.astype(np.float32))
        for st, t in enumerate((th, tl)):
            # [s, NSUB, r] -> [s, G, 2, r]
            w[:, :, sb, st, :, :] = _toep256(
                t.astype(np.float32), shift).reshape(
                B, G, 2, B).astype(ml_dtypes.float8_e4m3)
    return w


def _build_x_ops(data):
    """Global polyphase col matrices: for each (op, stream) a [B, WTOT] fp8
    matrix, half-block k at col PADL + k."""
    import ml_dtypes
    NB = N // B                                   # 16384 full blocks
    blocks = data.reshape(NB, B)
    KHALF = NCORES * HB_PER_CORE                  # 8320 half-blocks
    xe = np.zeros((KHALF, B), np.float32)
    xo = np.zeros((KHALF, B), np.float32)
    xe[:NB // 2] = blocks[0::2]
    xo[:NB // 2] = blocks[1::2]
    xs = xe + xo
    PADL = 132
    WTOT = PADL + KHALF + 8
    out = []
    for op in (xe, xo, xs):
        hi = _q8(op * SX)
        lo = _q8(op * SX - hi.astype(np.float32))
        mats = []
        for strm in (hi, lo):
            g = np.zeros((B, WTOT), dtype=ml_dtypes.float8_e4m3)
            g[:, PADL:PADL + KHALF] = strm.T
            mats.append(g)
        out.append(mats)
    return out, PADL


def _shard(xops, PADL, i):
    """Core i input: [B, 3, 2, 2, XW] fp8."""
    import ml_dtypes
    K0 = i * HB_PER_CORE
    x = np.empty((B, 3, 2, 2, XW), dtype=ml_dtypes.float8_e4m3)
    for sb in range(3):
        # xe/xs: plane_i[c] = op[K0 - 128 + c - i]
        # xo:    plane_i[c] = op[K0 - 129 + c - i]  (B computed shifted by 1)
        shift = 129 if sb == 1 else 128
        base = PADL + K0 - shift
        for st in range(2):
            g = xops[sb][st]
            x[:, sb, st, 0, :] = g[:, base:base + XW]
            x[:, sb, st, 1, :] = g[:, base - 1:base - 1 + XW]
    return x


def kernel(data, rir):
    global _NC_CACHE
    from concourse.bass_utils import run_bass_kernel_spmd

    data = np.asarray(data, dtype=np.float32).reshape(-1)
    h = np.asarray(rir, dtype=np.float32).reshape(-1)

    if _NC_CACHE is None:
        _NC_CACHE = _build_nc()
    nc = _NC_CACHE

    w = _build_weights(h)
    xops, PADL = _build_x_ops(data)
    in_maps = [{"x": _shard(xops, PADL, i), "w": w} for i in range(NCORES)]
    res = run_bass_kernel_spmd(nc, in_maps, core_ids=list(range(NCORES)))

    y = np.empty(NCORES * BLK_PER_CORE * B, np.float32)
    span = BLK_PER_CORE * B
    for i in range(NCORES):
        y[i * span:(i + 1) * span] = \
            res.results[i]["y"].reshape(B, BLK_PER_CORE).T.reshape(-1)
    return y[:NOUT]
